# revision 32
# baseline (speedup 1.0000x reference)
"""Multi-head attention TRN2 kernel (B=2, S=2048, D=1024, H=16).

Sharding (8 cores): B(2) x head-group(2) x query-block(2).
Each core: one batch b, 8 heads, 1024 query rows. The output projection
is a per-head-group partial sum; the host adds the two partials while
gathering (unshard step).

All on-chip compute is bf16 (fp8 anywhere in the q/k/v/E path measures
~3.5% output error per projection - the attention output is itself an
average of ~unit values, so elementwise quantization noise does NOT
average down relative to the signal).

The kernel is DVE-occupancy-bound (each DVE op pays an init+drain
overhead of ~150-200ns on top of its streaming time), so the design
minimizes DVE op count and keeps every op in its fastest perf mode:
- the post-exp (E-1) and *mask run once per 4-chunk GROUP as
  [128,4096] ops: tensor_scalar at 4x mode + tensor_tensor at 2x,
  amortizing the per-op overhead ~4x;
- the mask is DMA'd UN-duplicated and read with a stride-0 broadcast
  AP over the 2-head dim (halves mask DMA and SBUF);
- the denominator "+S" is accumulated into the PSUM ones-row by a
  K=1 f32r matmul (kills 16 DVE bias-adds);
- both heads' PV accumulate into ONE [65,2,512] PSUM tile, so the
  reciprocal runs once per pair on [1,1024] and ONE gpsimd
  partition_broadcast serves both heads;
- PSUM evacuations are split ACT/DVE (ACT is ~20% cheaper per PSUM
  read and has slack between exps; startup evacuations all go to ACT
  which is otherwise idle there).

Iteration order is nb-major (all 4 pairs at nb=0, then nb=1): all
kh/qh stay resident, the nb=0 output projection overlaps the nb=1
sweep, and only the last pair's tail is on the drain path. The qb=1
query projections are deferred to the (3,0) iteration, and the qT
transfer is split so only the qb=0 half is on the startup critical
path.

Softmax: the reference masks scores with -1e-9 (sic), so masked
positions contribute exp(-1e-9) == 1.0f exactly. PV weights are
a = (exp(s/8) - 1) * m; the "+1 everywhere" plane is restored from
HOST-precomputed V column sums (svh = (sum_tok v) @ Wv + S*bv) and an
appended ones-column on V gives the mask-dependent part of the
denominator.
"""

import sys

if "/opt/trn_rl_repo" not in sys.path:
    sys.path.insert(0, "/opt/trn_rl_repo")

import numpy as np
import ml_dtypes

import concourse.bass as bass
import concourse.tile as tile
from concourse import bacc, mybir
from concourse.bass_utils import run_bass_kernel_spmd

F32 = mybir.dt.float32
F32R = mybir.dt.float32r
BF16 = mybir.dt.bfloat16
AF = mybir.ActivationFunctionType
ALU = mybir.AluOpType

B, S, D, H = 2, 2048, 1024, 16
DK = 64
Q = 1024          # query rows per core
DH = 512          # head-group feature dims per core
NPAIR = 4         # head pairs per core
KC = S // 128     # 16 contraction chunks over k tokens
NG = KC // 4      # 4 chunk-groups per iteration (DVE ops run per group)
EC = D // 128     # 8 contraction chunks over model dim
QNB = Q // 512    # 2 query n-blocks
SNB = S // 512    # 4 khT n-blocks
HC = DH // 128    # 4 xT partition chunks

_PROGRAM = None


def _build_program():
    nc = bacc.Bacc("TRN2", debug=False, num_devices=8)

    qT = nc.dram_tensor("qT", [128, QNB, EC, 512], BF16, kind="ExternalInput")
    kT = nc.dram_tensor("kT", [128, SNB, EC, 512], BF16, kind="ExternalInput")
    vT = nc.dram_tensor("vT", [128, 4, EC, 512], BF16, kind="ExternalInput")
    maskT = nc.dram_tensor("maskT", [128, KC, QNB, 512], BF16,
                           kind="ExternalInput")
    svhv = nc.dram_tensor("svhv", [65, 8], F32, kind="ExternalInput")
    wqT = nc.dram_tensor("wqT", [128, EC, DH], BF16, kind="ExternalInput")
    wkT = nc.dram_tensor("wkT", [128, EC, DH], BF16, kind="ExternalInput")
    wvT = nc.dram_tensor("wvT", [128, EC, DH], BF16, kind="ExternalInput")
    woT = nc.dram_tensor("woT", [128, HC, D], BF16, kind="ExternalInput")
    bqv = nc.dram_tensor("bqv", [128, NPAIR], F32, kind="ExternalInput")
    bkv = nc.dram_tensor("bkv", [128, NPAIR], F32, kind="ExternalInput")
    bov = nc.dram_tensor("bov", [128, EC], F32, kind="ExternalInput")
    out = nc.dram_tensor("out", [D, Q], F32, kind="ExternalOutput")

    with tile.TileContext(nc) as tc:
        _emit(nc, tc, qT, kT, vT, maskT, wqT, wkT, wvT, woT, bqv, bkv, bov,
              svhv, out)
    nc.compile()
    return nc


def _emit(nc, tc, qT, kT, vT, maskT, wqT, wkT, wvT, woT, bqv, bkv, bov, svhv,
          out):
    from contextlib import ExitStack

    ctx = ExitStack()
    with ctx:
        consts = ctx.enter_context(tc.tile_pool(name="consts", bufs=1))
        big = ctx.enter_context(tc.tile_pool(name="big", bufs=1))
        work = ctx.enter_context(tc.tile_pool(name="work", bufs=2))
        pp = ctx.enter_context(tc.tile_pool(name="pp", bufs=2, space="PSUM"))
        pvp = ctx.enter_context(tc.tile_pool(name="pvp", bufs=1, space="PSUM"))
        scp = ctx.enter_context(tc.tile_pool(name="scp", bufs=2, space="PSUM"))
        kqd_cm = tc.tile_pool(name="kqd", bufs=1, side="right")
        kqd = kqd_cm.__enter__()
        vsp_cm = tc.tile_pool(name="vsp", bufs=1)
        vsp = vsp_cm.__enter__()

        # ---- small constants ----
        t_bq = consts.tile([128, NPAIR], F32)
        t_bk = consts.tile([128, NPAIR], F32)
        t_bo = consts.tile([128, EC], F32)
        nc.gpsimd.dma_start(out=t_bq, in_=bqv[:, :])
        nc.gpsimd.dma_start(out=t_bk, in_=bkv[:, :])
        nc.gpsimd.dma_start(out=t_bo, in_=bov[:, :])
        svh_sb = consts.tile([65, 8], F32)
        nc.gpsimd.dma_start(out=svh_sb, in_=svhv[:, :])
        ones_f0 = consts.tile([1, 64], F32)
        nc.vector.memset(ones_f0, 1.0)
        ones_r = consts.tile([1, 64], F32R)
        with nc.allow_low_precision(reason="f32r ones for denominator broadcast"):
            nc.vector.tensor_copy(out=ones_r, in_=ones_f0)
        # the K=1 "+S" matmul operands: [0..0,S](1,65) @ ones(1,512) -
        # rows 0:64 accumulate +0 (harmless), the ones-row gets +S
        ones512_f = consts.tile([1, 512], F32)
        nc.vector.memset(ones512_f, 1.0)
        ones512_r = consts.tile([1, 512], F32R)
        sv_f = consts.tile([1, 65], F32)
        nc.vector.memset(sv_f, 0.0)
        nc.vector.memset(sv_f[:, 64:65], float(S))
        sv_r = consts.tile([1, 65], F32R)
        with nc.allow_low_precision(reason="f32r +S denominator matmul"):
            nc.vector.tensor_copy(out=ones512_r, in_=ones512_f)
            nc.vector.tensor_copy(out=sv_r, in_=sv_f)
        # warm the ACT exp table set (~2.7us load) while ACT is idle at
        # startup instead of on the first real exp of the attention loop
        warm = consts.tile([1, 16], F32)
        nc.vector.memset(warm, 0.0)
        warm2 = consts.tile([1, 16], BF16)
        nc.scalar.activation(out=warm2, in_=warm, func=AF.Exp, scale=1.0)

        # ---- resident activation tensors (all pairs live: nb-major) ----
        khTs = {p: big.tile([128, S], BF16, name=f"khT{p}") for p in range(NPAIR)}
        qhTs = {p: big.tile([128, Q], BF16, name=f"qhT{p}") for p in range(NPAIR)}
        # one big vh tile: ONE memset covers every ones-column
        vh_aug = big.tile([128, KC, 8, 65], BF16, name="vha")
        nc.vector.memset(vh_aug, 1.0)
        # mask un-duplicated; the group (E-1)*m op reads it with a
        # stride-0 broadcast over the 2-head dim
        mbt = big.tile([128, KC, QNB, 1, 512], BF16, name="mbt")
        xT = [big.tile([128, Q], BF16, name=f"xT{i}") for i in range(HC)]

        wvf = vsp.tile([128, EC, DH], BF16, name="wvf")
        kTf = kqd.tile([128, SNB, EC, 512], BF16, name="kTf")
        qTf0 = kqd.tile([128, EC, 512], BF16, name="qTf0")
        qTfs = [qTf0, None]
        wkf = kqd.tile([128, EC, DH], BF16, name="wkf")
        wqf = kqd.tile([128, EC, DH], BF16, name="wqf")

        # V input quarters rotate through 2 buffers (quarters 0/1 are
        # consumed at startup, 2/3 inside iteration (0,0)); their DMAs
        # ride the gpsimd queue so the WAR wait on buffer reuse doesn't
        # stall the mask transfers on the sync queue.
        vqs = [vsp.tile([128, EC, 512], BF16, tag="vq", bufs=2, name=f"vq{t}")
               for t in range(4)]

        # inputs in PE-need order; transfers serialize per queue, so
        # queue order IS arrival order
        # two hw DMA queues, balanced ~9MB/7MB, ordered by PE need time;
        # vq quarter 3 reuses quarter 1's buffer, and its WAR wait rides
        # the scalar queue where the later masks can afford the stall
        nc.sync.dma_start(out=wkf, in_=wkT[:, :, :])
        nc.sync.dma_start(out=kTf[:, 0], in_=kT[:, 0])
        nc.sync.dma_start(out=kTf[:, 1], in_=kT[:, 1])
        nc.sync.dma_start(out=kTf[:, 2], in_=kT[:, 2])
        nc.sync.dma_start(out=wvf, in_=wvT[:, :, :])
        nc.sync.dma_start(out=vqs[0], in_=vT[:, 0])
        nc.sync.dma_start(out=kTf[:, 3], in_=kT[:, 3])
        nc.sync.dma_start(out=vqs[1], in_=vT[:, 1])
        nc.sync.dma_start(out=vqs[2], in_=vT[:, 2])
        nc.scalar.dma_start(out=wqf, in_=wqT[:, :, :])
        nc.scalar.dma_start(out=qTf0, in_=qT[:, 0])
        nc.scalar.dma_start(out=mbt[:, 0:4, :, 0, :], in_=maskT[:, 0:4])
        nc.scalar.dma_start(out=mbt[:, 4:8, :, 0, :], in_=maskT[:, 4:8])
        nc.scalar.dma_start(out=vqs[3], in_=vT[:, 3])
        nc.scalar.dma_start(out=mbt[:, 8:12, :, 0, :], in_=maskT[:, 8:12])
        nc.scalar.dma_start(out=mbt[:, 12:16, :, 0, :], in_=maskT[:, 12:16])

        def proj_tile(p, j, evac_act):
            if j < SNB:
                sb = j
                ps = pp.tile([128, 512], F32, tag="pp", name=f"khps{p}_{sb}")
                for ec in range(EC):
                    nc.tensor.matmul(
                        ps[:, :], wkf[:, ec, p * 128:(p + 1) * 128],
                        kTf[:, sb, ec, :],
                        start=(ec == 0), stop=(ec == EC - 1))
                dst = khTs[p][:, sb * 512:(sb + 1) * 512]
                bias = t_bk[:, p:p + 1]
            else:
                qb = j - SNB
                ps = pp.tile([128, 512], F32, tag="pp", name=f"qhps{p}_{qb}")
                for ec in range(EC):
                    nc.tensor.matmul(
                        ps[:, :], wqf[:, ec, p * 128:(p + 1) * 128],
                        qTfs[qb][:, ec, :],
                        start=(ec == 0), stop=(ec == EC - 1))
                dst = qhTs[p][:, qb * 512:(qb + 1) * 512]
                bias = t_bq[:, p:p + 1]
            if evac_act:
                nc.scalar.activation(out=dst, in_=ps[:, :], func=AF.Identity,
                                     bias=bias, scale=1.0)
            else:
                nc.vector.tensor_scalar(
                    out=dst, in0=ps[:, :], scalar1=bias, scalar2=None,
                    op0=ALU.add)

        def emit_vproj_qtr(qtr, evac_act):
            vqf = vqs[qtr]
            for si in range(4):
                sc = qtr * 4 + si
                ps = pp.tile([128, 512], F32, tag="pp", name=f"vps{sc}")
                for ec in range(EC):
                    nc.tensor.matmul(
                        ps[:, :], vqf[:, ec, si * 128:(si + 1) * 128],
                        wvf[:, ec, :],
                        start=(ec == 0), stop=(ec == EC - 1))
                src = ps.rearrange("p (h d) -> p h d", h=8)
                dst = vh_aug[:, sc, :, 0:64]
                if evac_act:
                    nc.scalar.copy(out=dst, in_=src)
                else:
                    nc.vector.tensor_copy(out=dst, in_=src)

        # startup: the minimal prefix for iteration (0,0) - pair-0
        # projections and V quarter 0 in DMA arrival order (quarters
        # 1..3 are emitted inside (0,0)'s first groups); ACT is idle
        # here so it takes every evac
        proj_tile(0, 0, True)
        proj_tile(0, 4, True)
        proj_tile(0, 1, True)
        proj_tile(0, 2, True)
        emit_vproj_qtr(0, evac_act=True)
        proj_tile(0, 3, True)

        # ---- attention ----
        pending_tail = [None]
        wob = [None]

        def emit_outproj_dc(nb, dc, evac_act):
            ps = pp.tile([128, 512], F32, tag="pp", name=f"ops{dc}_{nb}")
            for hc in range(HC):
                nc.tensor.matmul(
                    ps[:, :], wob[0][:, hc, dc * 128:(dc + 1) * 128],
                    xT[hc][:, nb * 512:(nb + 1) * 512],
                    start=(hc == 0), stop=(hc == HC - 1))
            o_sb = work.tile([128, 512], F32, tag="osb", bufs=2,
                             name=f"osb{dc}_{nb}")
            if evac_act:
                nc.scalar.add(out=o_sb, in_=ps[:, :], add=t_bo[:, dc:dc + 1])
            else:
                nc.vector.tensor_scalar(
                    out=o_sb, in0=ps[:, :], scalar1=t_bo[:, dc:dc + 1],
                    scalar2=None, op0=ALU.add)
            nc.sync.dma_start(
                out=out[dc * 128:(dc + 1) * 128, nb * 512:(nb + 1) * 512],
                in_=o_sb)

        def run_iteration(p, nb, vproj=False, projpair=None, qb1pair=False,
                          outp0=False, last=False):
            # both heads accumulate into ONE 2-bank PSUM tile; row 64 is
            # the ones-column (denominator) row
            pv = pvp.tile([65, 2, 512], F32, tag="pv", name=f"pv_{p}{nb}")
            EQs = {}

            def emit_scores(kc, EQ):
                sc_ps = scp.tile([128, 2, 512], F32, tag="sc",
                                 name=f"sc_{p}_{nb}_{kc}")
                nc.tensor.matmul(
                    sc_ps[:, 0, :],
                    khTs[p][0:64, kc * 128:(kc + 1) * 128],
                    qhTs[p][0:64, nb * 512:(nb + 1) * 512],
                    start=True, stop=True)
                nc.tensor.matmul(
                    sc_ps[:, 1, :],
                    khTs[p][64:128, kc * 128:(kc + 1) * 128],
                    qhTs[p][64:128, nb * 512:(nb + 1) * 512],
                    start=True, stop=True, tile_position=(64, 0))
                # exp straight from PSUM into this chunk's slot of the
                # group tile
                nc.scalar.activation(
                    out=EQ[:, kc % 4].rearrange("p h q -> p (h q)"),
                    in_=sc_ps.rearrange("p h q -> p (h q)"),
                    func=AF.Exp, scale=0.125)

            def emit_group_mask(g, EQ, half):
                # (E-1) at DVE 4x + *mask at 2x, in 2-chunk halves so the
                # first PV pair of a group unblocks right after its 2 exps
                sl = EQ[:, 2 * half:2 * half + 2]
                slf = sl.rearrange("a b h q -> a (b h q)")
                nc.vector.tensor_scalar(
                    out=slf, in0=slf, scalar1=-1.0, scalar2=None, op0=ALU.add)
                c = 4 * g + 2 * half
                nc.vector.tensor_tensor(
                    out=sl, in0=sl,
                    in1=mbt[:, c:c + 2, nb].to_broadcast([128, 2, 2, 512]),
                    op=ALU.mult)

            def emit_pv(kc, EQ):
                # pv row 0 is the ones-column (denominator) row so the
                # reciprocal later reads a partition-0 AP (the custom
                # DVE ucode ignores nonzero base partitions)
                for hh in range(2):
                    nc.tensor.matmul(
                        pv[:, hh, :], vh_aug[:, kc, 2 * p + hh, :],
                        EQ[:, kc % 4, hh, :],
                        start=(kc == 0), stop=False)

            # per-group pipeline: scores+exp of group g interleaved (in
            # PE order) with PV of g-1, so the PE always has PV work
            # while it waits on score-PSUM banks; the DVE mask ops run
            # in 2-chunk halves behind the exps
            for g in range(NG + 1):
                EQ = None
                if g < NG:
                    EQ = work.tile([128, 4, 2, 512], BF16, tag="EQ", bufs=2,
                                   name=f"EQ{p}_{nb}_{g}")
                    if vproj and g in (0, 1, 2):
                        emit_vproj_qtr(1 + g, evac_act=(g == 0))
                EQprev = EQs.pop(g - 1) if g >= 1 else None
                for i in range(4):
                    if g < NG:
                        emit_scores(4 * g + i, EQ)
                        if i == 1:
                            emit_group_mask(g, EQ, 0)
                        elif i == 3:
                            emit_group_mask(g, EQ, 1)
                    if g == 1 and i == 0 and pending_tail[0] is not None:
                        pending_tail[0]()
                        pending_tail[0] = None
                    if EQprev is not None:
                        emit_pv(4 * (g - 1) + i, EQprev)
                if g < NG:
                    EQs[g] = EQ
                if projpair is not None and 1 <= g:
                    # 5 tiles (kh x4 + qh qb0) spread over groups 1..4
                    for j in {1: [0], 2: [1, 2], 3: [3], 4: [4]}[g]:
                        proj_tile(projpair, j, evac_act=(j % 2 == 0))
                if qb1pair and 1 <= g <= 4:
                    # deferred qb=1 projections for all pairs
                    proj_tile(g - 1, 5, evac_act=(g % 2 == 0))
                if outp0 and 1 <= g:
                    emit_outproj_dc(0, (0 if p == 1 else 4) + g - 1,
                                    evac_act=False)

            # close the accumulation with the "+S" ones-row matmuls, then
            # evacuate both heads (ACT + DVE) so the pv banks free fast
            for hh in range(2):
                nc.tensor.matmul(
                    pv[:, hh, :], sv_r, ones512_r,
                    start=False, stop=True, skip_group_check=True)
            # the denominator rows go to a partition-0 tile (the custom
            # reciprocal ucode ignores nonzero base partitions) via ACT,
            # which reads PSUM cheaply
            pvn = work.tile([64, 2, 512], BF16, tag="pvn", bufs=2,
                            name=f"pvn{p}{nb}")
            den2 = work.tile([1, 2, 512], F32, tag="den2", bufs=1,
                             name=f"den2{p}{nb}")
            nc.scalar.copy(out=den2.rearrange("p h q -> p (h q)"),
                           in_=pv[64:65, :, :].rearrange("p h q -> p (h q)"))
            nc.scalar.copy(out=pvn[:, 0, :], in_=pv[0:64, 0, :])
            nc.vector.tensor_copy(out=pvn[:, 1, :], in_=pv[0:64, 1, :])

            def tail():
                h0 = 2 * p
                # ONE reciprocal for both heads' denominator rows
                rcp_f = work.tile([1, 2, 512], F32, tag="rcpf", bufs=1,
                                  name=f"rcpf{p}{nb}")
                nc.vector.reciprocal_approx_fast(
                    out=rcp_f.rearrange("p h q -> p (h q)"),
                    in_=den2.rearrange("p h q -> p (h q)"))
                if last:
                    # endgame: the PE is idle and this chain gates the
                    # nb=1 output projection - broadcast via K=1 matmuls
                    # instead of the slower gpsimd path
                    brs = []
                    for hh in range(2):
                        rcpr = work.tile([1, 512], F32R, tag="rcpr", bufs=1,
                                         name=f"rcpr{p}{nb}{hh}")
                        with nc.allow_low_precision(
                                reason="f32r denominator broadcast"):
                            nc.vector.tensor_copy(out=rcpr,
                                                  in_=rcp_f[:, hh, :])
                        br_ps = pp.tile([64, 512], F32, tag="pp",
                                        name=f"br{p}{nb}{hh}")
                        nc.tensor.matmul(br_ps[:, :], ones_r, rcpr,
                                         start=True, stop=True)
                        brs.append(br_ps)
                    for hh in range(2):
                        h = h0 + hh
                        nc.vector.scalar_tensor_tensor(
                            out=xT[h // 2][(h % 2) * 64:(h % 2) * 64 + 64,
                                           nb * 512:(nb + 1) * 512],
                            in0=pvn[:, hh, :],
                            scalar=svh_sb[0:64, h:h + 1],
                            in1=brs[hh][:, :], op0=ALU.add, op1=ALU.mult)
                    return
                # ONE gpsimd broadcast serves both heads
                br = work.tile([64, 2, 512], F32, tag="brs", bufs=1,
                               name=f"brs{p}{nb}")
                nc.gpsimd.partition_broadcast(
                    br.rearrange("p h q -> p (h q)"),
                    rcp_f.rearrange("p h q -> p (h q)"))
                for hh in range(2):
                    h = h0 + hh
                    nc.vector.scalar_tensor_tensor(
                        out=xT[h // 2][(h % 2) * 64:(h % 2) * 64 + 64,
                                       nb * 512:(nb + 1) * 512],
                        in0=pvn[:, hh, :], scalar=svh_sb[0:64, h:h + 1],
                        in1=br[:, hh, :], op0=ALU.add, op1=ALU.mult)

            pending_tail[0] = tail

        for nb in range(QNB):
            for p in range(NPAIR):
                run_iteration(
                    p, nb,
                    vproj=(nb == 0 and p == 0),
                    projpair=(p + 1 if (nb == 0 and p < NPAIR - 1) else None),
                    qb1pair=(nb == 0 and p == NPAIR - 1),
                    outp0=(nb == 1 and p in (1, 2)),
                    last=(nb == 1 and p == NPAIR - 1))
                if nb == 0 and p == 0:
                    # V inputs are dead once (0,0)'s vproj is emitted;
                    # reuse the space for the qb=1 queries + wob
                    vsp_cm.__exit__(None, None, None)
                    late = ctx.enter_context(tc.tile_pool(name="late", bufs=1))
                    qTf1 = late.tile([128, EC, 512], BF16, name="qTf1")
                    qTfs[1] = qTf1
                    nc.sync.dma_start(out=qTf1, in_=qT[:, 1])
                    wof = late.tile([128, HC, D], BF16, name="wof")
                    wob[0] = wof
                    nc.sync.dma_start(out=wof, in_=woT[:, :, :])
                if nb == 0 and p == 2:
                    # kq inputs/weights die after proj_pair(3) is emitted
                    kqd_cm.__exit__(None, None, None)
        pending_tail[0]()
        pending_tail[0] = None
        for dc in range(EC):
            emit_outproj_dc(1, dc, evac_act=True)


def _get_program():
    global _PROGRAM
    if _PROGRAM is None:
        _PROGRAM = _build_program()
    return _PROGRAM


def kernel(q, k, v, mask, Wq, bq, Wk, bk, Wv, bv, Wo, bo, _trace=False):
    bf16 = ml_dtypes.bfloat16
    q = np.asarray(q, np.float32)
    k = np.asarray(k, np.float32)
    v = np.asarray(v, np.float32)
    Wq = np.asarray(Wq, np.float32)
    Wk = np.asarray(Wk, np.float32)
    Wv = np.asarray(Wv, np.float32)
    Wo = np.asarray(Wo, np.float32)
    bq = np.asarray(bq, np.float32)
    bk = np.asarray(bk, np.float32)
    bv = np.asarray(bv, np.float32)
    bo = np.asarray(bo, np.float32)
    mask_f = np.asarray(mask).astype(np.float32)

    nc = _get_program()

    def fold_ec(xT, inner):  # [D, N] -> [128, EC, N] views
        N = xT.shape[1]
        a = np.ascontiguousarray(xT.reshape(EC, 128, N).transpose(1, 0, 2))
        return a.astype(bf16).reshape((128,) + inner)

    # qT: [D, Q] -> [128, QNB, EC, 512] (query-block major)
    def fold_q(xT):
        a = xT.reshape(EC, 128, QNB, 512).transpose(1, 2, 0, 3)
        return np.ascontiguousarray(a).astype(bf16)

    # kT: [D, S] -> [128, SNB, EC, 512] (512-col block major)
    kT_b = [np.ascontiguousarray(
        k[b].T.reshape(EC, 128, SNB, 512).transpose(1, 2, 0, 3)).astype(bf16)
        for b in range(B)]
    # vT: [D, S] -> [128, 4, EC, 512] (quarter major)
    vT_b = [np.ascontiguousarray(
        v[b].T.reshape(EC, 128, 4, 512).transpose(1, 2, 0, 3)).astype(bf16)
        for b in range(B)]
    # mask un-duplicated: [128, KC, QNB, 512]
    def build_mask(mT):  # mT: [S, Q] 0/1 float
        a = mT.reshape(KC, 128, QNB, 512).transpose(1, 0, 2, 3)
        return np.ascontiguousarray(a).astype(bf16)
    mask_b = [[build_mask(mask_f[b, 0, sq * Q:(sq + 1) * Q, :].T)
               for sq in range(2)] for b in range(B)]
    # host-side V column sums: svh[0:64, h] = (sum_tok v) @ Wv_h + S*bv_h
    svh_b = []
    for b in range(B):
        vsum = v[b].sum(0)  # [D]
        per_g = []
        for hg in range(2):
            hsl_g = slice(hg * DH, (hg + 1) * DH)
            s = vsum @ Wv[hsl_g, :].T + S * bv[hsl_g]  # [DH]
            arr = np.zeros((65, 8), np.float32)
            arr[0:64, :] = s.reshape(8, 64).T
            arr[64, :] = S
            per_g.append(np.ascontiguousarray(arr))
        svh_b.append(per_g)

    in_maps = []
    for c in range(8):
        b, hg, sq = c // 4, (c // 2) % 2, c % 2
        hsl = slice(hg * DH, (hg + 1) * DH)
        in_maps.append({
            "qT": fold_q(q[b, sq * Q:(sq + 1) * Q, :].T),
            "kT": kT_b[b],
            "vT": vT_b[b],
            "maskT": mask_b[b][sq],
            "svhv": svh_b[b][hg],
            "wqT": fold_ec(Wq.T[:, hsl], (EC, DH)),
            "wkT": fold_ec(Wk.T[:, hsl], (EC, DH)),
            "wvT": fold_ec(Wv.T[:, hsl], (EC, DH)),
            "woT": np.ascontiguousarray(
                Wo.T[hsl, :].reshape(HC, 128, D).transpose(1, 0, 2)).astype(bf16),
            "bqv": np.ascontiguousarray(bq[hsl].reshape(NPAIR, 128).T),
            "bkv": np.ascontiguousarray(bk[hsl].reshape(NPAIR, 128).T),
            "bov": np.ascontiguousarray(
                ((bo if hg == 0 else np.zeros_like(bo))
                 + Wo[:, hsl] @ bv[hsl]).reshape(EC, 128).T),
        })

    kw = {}
    if _trace:
        kw = dict(trace=True, trace_cores=list(range(8)))
    res = run_bass_kernel_spmd(nc, in_maps, core_ids=list(range(8)), **kw)
    kernel._last_res = res

    outp = np.empty((B, S, D), np.float32)
    for b in range(B):
        for sq in range(2):
            c0 = b * 4 + sq
            c1 = b * 4 + 2 + sq
            outp[b, sq * Q:(sq + 1) * Q, :] = (
                res.results[c0]["out"] + res.results[c1]["out"]).T
    if _trace:
        return outp, res
    return outp


# revision 35
# speedup vs baseline: 1.0038x; 1.0038x over previous
"""Multi-head attention TRN2 kernel (B=2, S=2048, D=1024, H=16).

Sharding (8 cores): B(2) x head-group(2) x query-block(2).
Each core: one batch b, 8 heads, 1024 query rows. The output projection
is a per-head-group partial sum; the host adds the two partials while
gathering (unshard step).

All on-chip compute is bf16 (fp8 anywhere in the q/k/v/E path measures
~3.5% output error per projection - the attention output is itself an
average of ~unit values, so elementwise quantization noise does NOT
average down relative to the signal).

The kernel is DVE-occupancy-bound (each DVE op pays an init+drain
overhead of ~150-200ns on top of its streaming time), so the design
minimizes DVE op count and keeps every op in its fastest perf mode:
- the post-exp (E-1) and *mask run once per 4-chunk GROUP as
  [128,4096] ops: tensor_scalar at 4x mode + tensor_tensor at 2x,
  amortizing the per-op overhead ~4x;
- the mask is DMA'd UN-duplicated and read with a stride-0 broadcast
  AP over the 2-head dim (halves mask DMA and SBUF);
- the denominator "+S" is accumulated into the PSUM ones-row by a
  K=1 f32r matmul (kills 16 DVE bias-adds);
- both heads' PV accumulate into ONE [65,2,512] PSUM tile, so the
  reciprocal runs once per pair on [1,1024] and ONE gpsimd
  partition_broadcast serves both heads;
- PSUM evacuations are split ACT/DVE (ACT is ~20% cheaper per PSUM
  read and has slack between exps; startup evacuations all go to ACT
  which is otherwise idle there).

Iteration order is nb-major (all 4 pairs at nb=0, then nb=1): all
kh/qh stay resident, the nb=0 output projection overlaps the nb=1
sweep, and only the last pair's tail is on the drain path. The qb=1
query projections are deferred to the (3,0) iteration, and the qT
transfer is split so only the qb=0 half is on the startup critical
path.

Softmax: the reference masks scores with -1e-9 (sic), so masked
positions contribute exp(-1e-9) == 1.0f exactly. PV weights are
a = (exp(s/8) - 1) * m; the "+1 everywhere" plane is restored from
HOST-precomputed V column sums (svh = (sum_tok v) @ Wv + S*bv) and an
appended ones-column on V gives the mask-dependent part of the
denominator.
"""

import sys

if "/opt/trn_rl_repo" not in sys.path:
    sys.path.insert(0, "/opt/trn_rl_repo")

import numpy as np
import ml_dtypes

import concourse.bass as bass
import concourse.tile as tile
from concourse import bacc, mybir
from concourse.bass_utils import run_bass_kernel_spmd

F32 = mybir.dt.float32
F32R = mybir.dt.float32r
BF16 = mybir.dt.bfloat16
AF = mybir.ActivationFunctionType
ALU = mybir.AluOpType

B, S, D, H = 2, 2048, 1024, 16
DK = 64
Q = 1024          # query rows per core
DH = 512          # head-group feature dims per core
NPAIR = 4         # head pairs per core
KC = S // 128     # 16 contraction chunks over k tokens
NG = KC // 4      # 4 chunk-groups per iteration (DVE ops run per group)
EC = D // 128     # 8 contraction chunks over model dim
QNB = Q // 512    # 2 query n-blocks
SNB = S // 512    # 4 khT n-blocks
HC = DH // 128    # 4 xT partition chunks

_PROGRAM = None


def _build_program():
    nc = bacc.Bacc("TRN2", debug=False, num_devices=8)

    qT = nc.dram_tensor("qT", [128, QNB, EC, 512], BF16, kind="ExternalInput")
    kT = nc.dram_tensor("kT", [128, SNB, EC, 512], BF16, kind="ExternalInput")
    vT = nc.dram_tensor("vT", [128, 4, EC, 512], BF16, kind="ExternalInput")
    maskT = nc.dram_tensor("maskT", [128, KC, QNB, 512], BF16,
                           kind="ExternalInput")
    svhv = nc.dram_tensor("svhv", [65, 8], F32, kind="ExternalInput")
    wqT = nc.dram_tensor("wqT", [128, EC, DH], BF16, kind="ExternalInput")
    wkT = nc.dram_tensor("wkT", [128, EC, DH], BF16, kind="ExternalInput")
    wvT = nc.dram_tensor("wvT", [128, EC, DH], BF16, kind="ExternalInput")
    woT = nc.dram_tensor("woT", [128, HC, D], BF16, kind="ExternalInput")
    bqv = nc.dram_tensor("bqv", [128, NPAIR], F32, kind="ExternalInput")
    bkv = nc.dram_tensor("bkv", [128, NPAIR], F32, kind="ExternalInput")
    bov = nc.dram_tensor("bov", [128, EC], F32, kind="ExternalInput")
    out = nc.dram_tensor("out", [D, Q], F32, kind="ExternalOutput")

    with tile.TileContext(nc) as tc:
        _emit(nc, tc, qT, kT, vT, maskT, wqT, wkT, wvT, woT, bqv, bkv, bov,
              svhv, out)
    nc.compile()
    return nc


def _emit(nc, tc, qT, kT, vT, maskT, wqT, wkT, wvT, woT, bqv, bkv, bov, svhv,
          out):
    from contextlib import ExitStack

    ctx = ExitStack()
    with ctx:
        consts = ctx.enter_context(tc.tile_pool(name="consts", bufs=1))
        big = ctx.enter_context(tc.tile_pool(name="big", bufs=1))
        work = ctx.enter_context(tc.tile_pool(name="work", bufs=2))
        pp = ctx.enter_context(tc.tile_pool(name="pp", bufs=2, space="PSUM"))
        pvp = ctx.enter_context(tc.tile_pool(name="pvp", bufs=1, space="PSUM"))
        scp = ctx.enter_context(tc.tile_pool(name="scp", bufs=2, space="PSUM"))
        kqd_cm = tc.tile_pool(name="kqd", bufs=1, side="right")
        kqd = kqd_cm.__enter__()
        vsp_cm = tc.tile_pool(name="vsp", bufs=1)
        vsp = vsp_cm.__enter__()

        # ---- small constants ----
        t_bq = consts.tile([128, NPAIR], F32)
        t_bk = consts.tile([128, NPAIR], F32)
        t_bo = consts.tile([128, EC], F32)
        nc.gpsimd.dma_start(out=t_bq, in_=bqv[:, :])
        nc.gpsimd.dma_start(out=t_bk, in_=bkv[:, :])
        nc.gpsimd.dma_start(out=t_bo, in_=bov[:, :])
        svh_sb = consts.tile([65, 8], F32)
        nc.gpsimd.dma_start(out=svh_sb, in_=svhv[:, :])
        ones_f0 = consts.tile([1, 64], F32)
        nc.vector.memset(ones_f0, 1.0)
        ones_r = consts.tile([1, 64], F32R)
        with nc.allow_low_precision(reason="f32r ones for denominator broadcast"):
            nc.vector.tensor_copy(out=ones_r, in_=ones_f0)
        # the K=1 "+S" matmul operands: [0..0,S](1,65) @ ones(1,512) -
        # rows 0:64 accumulate +0 (harmless), the ones-row gets +S
        ones512_f = consts.tile([1, 512], F32)
        nc.vector.memset(ones512_f, 1.0)
        ones512_r = consts.tile([1, 512], F32R)
        sv_f = consts.tile([1, 65], F32)
        nc.vector.memset(sv_f, 0.0)
        nc.vector.memset(sv_f[:, 64:65], float(S))
        sv_r = consts.tile([1, 65], F32R)
        with nc.allow_low_precision(reason="f32r +S denominator matmul"):
            nc.vector.tensor_copy(out=ones512_r, in_=ones512_f)
            nc.vector.tensor_copy(out=sv_r, in_=sv_f)
        # warm the ACT exp table set (~2.7us load) while ACT is idle at
        # startup instead of on the first real exp of the attention loop
        warm = consts.tile([1, 16], F32)
        nc.vector.memset(warm, 0.0)
        warm2 = consts.tile([1, 16], BF16)
        nc.scalar.activation(out=warm2, in_=warm, func=AF.Exp, scale=1.0)

        # ---- resident activation tensors (all pairs live: nb-major) ----
        khTs = {p: big.tile([128, S], BF16, name=f"khT{p}") for p in range(NPAIR)}
        qhTs = {p: big.tile([128, Q], BF16, name=f"qhT{p}") for p in range(NPAIR)}
        # one big vh tile: ONE memset covers every ones-column
        vh_aug = big.tile([128, KC, 8, 65], BF16, name="vha")
        nc.vector.memset(vh_aug, 1.0)
        # mask un-duplicated; the group (E-1)*m op reads it with a
        # stride-0 broadcast over the 2-head dim
        mbt = big.tile([128, KC, QNB, 1, 512], BF16, name="mbt")
        xT = [big.tile([128, Q], BF16, name=f"xT{i}") for i in range(HC)]

        wvf = vsp.tile([128, EC, DH], BF16, name="wvf")
        kTf = kqd.tile([128, SNB, EC, 512], BF16, name="kTf")
        qTf0 = kqd.tile([128, EC, 512], BF16, name="qTf0")
        qTfs = [qTf0, None]
        wkf = kqd.tile([128, EC, DH], BF16, name="wkf")
        wqf = kqd.tile([128, EC, DH], BF16, name="wqf")

        # V input quarters rotate through 2 buffers (quarters 0/1 are
        # consumed at startup, 2/3 inside iteration (0,0)); their DMAs
        # ride the gpsimd queue so the WAR wait on buffer reuse doesn't
        # stall the mask transfers on the sync queue.
        vqs = [vsp.tile([128, EC, 512], BF16, tag="vq", bufs=2, name=f"vq{t}")
               for t in range(4)]

        # inputs in PE-need order; transfers serialize per queue, so
        # queue order IS arrival order
        # two hw DMA queues, balanced ~9MB/7MB, ordered by PE need time;
        # vq quarter 3 reuses quarter 1's buffer, and its WAR wait rides
        # the scalar queue where the later masks can afford the stall
        nc.sync.dma_start(out=wkf, in_=wkT[:, :, :])
        nc.sync.dma_start(out=kTf[:, 0], in_=kT[:, 0])
        nc.sync.dma_start(out=kTf[:, 1], in_=kT[:, 1])
        nc.sync.dma_start(out=kTf[:, 2], in_=kT[:, 2])
        nc.sync.dma_start(out=wvf, in_=wvT[:, :, :])
        nc.sync.dma_start(out=vqs[0], in_=vT[:, 0])
        nc.sync.dma_start(out=kTf[:, 3], in_=kT[:, 3])
        nc.sync.dma_start(out=vqs[1], in_=vT[:, 1])
        nc.sync.dma_start(out=vqs[2], in_=vT[:, 2])
        nc.scalar.dma_start(out=wqf, in_=wqT[:, :, :])
        nc.scalar.dma_start(out=qTf0, in_=qT[:, 0])
        nc.scalar.dma_start(out=mbt[:, 0:4, :, 0, :], in_=maskT[:, 0:4])
        nc.scalar.dma_start(out=mbt[:, 4:8, :, 0, :], in_=maskT[:, 4:8])
        nc.scalar.dma_start(out=vqs[3], in_=vT[:, 3])
        nc.scalar.dma_start(out=mbt[:, 8:12, :, 0, :], in_=maskT[:, 8:12])
        nc.scalar.dma_start(out=mbt[:, 12:16, :, 0, :], in_=maskT[:, 12:16])

        def proj_tile(p, j, evac_act):
            if j < SNB:
                sb = j
                ps = pp.tile([128, 512], F32, tag="pp", name=f"khps{p}_{sb}")
                for ec in range(EC):
                    nc.tensor.matmul(
                        ps[:, :], wkf[:, ec, p * 128:(p + 1) * 128],
                        kTf[:, sb, ec, :],
                        start=(ec == 0), stop=(ec == EC - 1))
                dst = khTs[p][:, sb * 512:(sb + 1) * 512]
                bias = t_bk[:, p:p + 1]
            else:
                qb = j - SNB
                ps = pp.tile([128, 512], F32, tag="pp", name=f"qhps{p}_{qb}")
                for ec in range(EC):
                    nc.tensor.matmul(
                        ps[:, :], wqf[:, ec, p * 128:(p + 1) * 128],
                        qTfs[qb][:, ec, :],
                        start=(ec == 0), stop=(ec == EC - 1))
                dst = qhTs[p][:, qb * 512:(qb + 1) * 512]
                bias = t_bq[:, p:p + 1]
            if evac_act:
                nc.scalar.activation(out=dst, in_=ps[:, :], func=AF.Identity,
                                     bias=bias, scale=1.0)
            else:
                nc.vector.tensor_scalar(
                    out=dst, in0=ps[:, :], scalar1=bias, scalar2=None,
                    op0=ALU.add)

        def emit_vproj_qtr(qtr, evac_act):
            vqf = vqs[qtr]
            for si in range(4):
                sc = qtr * 4 + si
                ps = pp.tile([128, 512], F32, tag="pp", name=f"vps{sc}")
                for ec in range(EC):
                    nc.tensor.matmul(
                        ps[:, :], vqf[:, ec, si * 128:(si + 1) * 128],
                        wvf[:, ec, :],
                        start=(ec == 0), stop=(ec == EC - 1))
                src = ps.rearrange("p (h d) -> p h d", h=8)
                dst = vh_aug[:, sc, :, 0:64]
                if evac_act:
                    nc.scalar.copy(out=dst, in_=src)
                else:
                    nc.vector.tensor_copy(out=dst, in_=src)

        # startup: the minimal prefix for iteration (0,0) - pair-0
        # projections and V quarter 0 in DMA arrival order (quarters
        # 1..3 are emitted inside (0,0)'s first groups); ACT is idle
        # here so it takes every evac
        proj_tile(0, 0, True)
        proj_tile(0, 4, True)
        proj_tile(0, 1, True)
        proj_tile(0, 2, True)
        emit_vproj_qtr(0, evac_act=True)
        proj_tile(0, 3, True)

        # ---- attention ----
        pending_tail = [None]
        wob = [None]

        def emit_outproj_dc(nb, dc, evac_act):
            ps = pp.tile([128, 512], F32, tag="pp", name=f"ops{dc}_{nb}")
            for hc in range(HC):
                nc.tensor.matmul(
                    ps[:, :], wob[0][:, hc, dc * 128:(dc + 1) * 128],
                    xT[hc][:, nb * 512:(nb + 1) * 512],
                    start=(hc == 0), stop=(hc == HC - 1))
            o_sb = work.tile([128, 512], F32, tag="osb", bufs=2,
                             name=f"osb{dc}_{nb}")
            if evac_act:
                nc.scalar.add(out=o_sb, in_=ps[:, :], add=t_bo[:, dc:dc + 1])
            else:
                nc.vector.tensor_scalar(
                    out=o_sb, in0=ps[:, :], scalar1=t_bo[:, dc:dc + 1],
                    scalar2=None, op0=ALU.add)
            nc.sync.dma_start(
                out=out[dc * 128:(dc + 1) * 128, nb * 512:(nb + 1) * 512],
                in_=o_sb)

        def run_iteration(p, nb, vproj=False, projpair=None, qb1pair=False,
                          outp0=False, last=False):
            # both heads accumulate into ONE 2-bank PSUM tile; row 64 is
            # the ones-column (denominator) row
            pv = pvp.tile([65, 2, 512], F32, tag="pv", name=f"pv_{p}{nb}")
            EQs = {}

            def emit_scores(kc, EQ):
                sc_ps = scp.tile([128, 2, 512], F32, tag="sc",
                                 name=f"sc_{p}_{nb}_{kc}")
                nc.tensor.matmul(
                    sc_ps[:, 0, :],
                    khTs[p][0:64, kc * 128:(kc + 1) * 128],
                    qhTs[p][0:64, nb * 512:(nb + 1) * 512],
                    start=True, stop=True)
                nc.tensor.matmul(
                    sc_ps[:, 1, :],
                    khTs[p][64:128, kc * 128:(kc + 1) * 128],
                    qhTs[p][64:128, nb * 512:(nb + 1) * 512],
                    start=True, stop=True, tile_position=(64, 0))
                # exp straight from PSUM into this chunk's slot of the
                # group tile
                nc.scalar.activation(
                    out=EQ[:, kc % 4].rearrange("p h q -> p (h q)"),
                    in_=sc_ps.rearrange("p h q -> p (h q)"),
                    func=AF.Exp, scale=0.125)

            def emit_group_mask(g, EQ, half):
                # (E-1) at DVE 4x + *mask at 2x, in 2-chunk halves so the
                # first PV pair of a group unblocks right after its 2 exps
                sl = EQ[:, 2 * half:2 * half + 2]
                slf = sl.rearrange("a b h q -> a (b h q)")
                nc.vector.tensor_scalar(
                    out=slf, in0=slf, scalar1=-1.0, scalar2=None, op0=ALU.add)
                c = 4 * g + 2 * half
                nc.vector.tensor_tensor(
                    out=sl, in0=sl,
                    in1=mbt[:, c:c + 2, nb].to_broadcast([128, 2, 2, 512]),
                    op=ALU.mult)

            def emit_pv(kc, EQ):
                # pv row 0 is the ones-column (denominator) row so the
                # reciprocal later reads a partition-0 AP (the custom
                # DVE ucode ignores nonzero base partitions)
                for hh in range(2):
                    nc.tensor.matmul(
                        pv[:, hh, :], vh_aug[:, kc, 2 * p + hh, :],
                        EQ[:, kc % 4, hh, :],
                        start=(kc == 0), stop=False)

            # per-group pipeline: scores+exp of group g interleaved (in
            # PE order) with PV of g-1, so the PE always has PV work
            # while it waits on score-PSUM banks; the DVE mask ops run
            # in 2-chunk halves behind the exps
            for g in range(NG + 1):
                EQ = None
                if g < NG:
                    EQ = work.tile([128, 4, 2, 512], BF16, tag="EQ", bufs=2,
                                   name=f"EQ{p}_{nb}_{g}")
                    if vproj and g in (0, 1, 2):
                        emit_vproj_qtr(1 + g, evac_act=(g == 0))
                EQprev = EQs.pop(g - 1) if g >= 1 else None
                for i in range(4):
                    if g < NG:
                        emit_scores(4 * g + i, EQ)
                        if i == 1:
                            emit_group_mask(g, EQ, 0)
                        elif i == 3:
                            emit_group_mask(g, EQ, 1)
                    if g == 2 and i == 0 and pending_tail[0] is not None:
                        # the previous iteration's tail drops into the
                        # DVE queue here, between group-1 and group-2
                        # mask ops, where it least delays PV unblocking
                        pending_tail[0]()
                        pending_tail[0] = None
                    if EQprev is not None:
                        emit_pv(4 * (g - 1) + i, EQprev)
                if g < NG:
                    EQs[g] = EQ
                if projpair is not None and 1 <= g:
                    # 5 tiles (kh x4 + qh qb0) spread over groups 1..4
                    for j in {1: [0], 2: [1, 2], 3: [3], 4: [4]}[g]:
                        proj_tile(projpair, j, evac_act=(j % 2 == 0))
                if qb1pair and 1 <= g <= 4:
                    # deferred qb=1 projections for all pairs
                    proj_tile(g - 1, 5, evac_act=(g % 2 == 0))
                if outp0 and 1 <= g:
                    emit_outproj_dc(0, (0 if p == 1 else 4) + g - 1,
                                    evac_act=False)

            # close the accumulation with the "+S" ones-row matmuls, then
            # evacuate both heads (ACT + DVE) so the pv banks free fast
            for hh in range(2):
                nc.tensor.matmul(
                    pv[:, hh, :], sv_r, ones512_r,
                    start=False, stop=True, skip_group_check=True)
            # the denominator rows go to a partition-0 tile (the custom
            # reciprocal ucode ignores nonzero base partitions) via ACT,
            # which reads PSUM cheaply
            pvn = work.tile([64, 2, 512], BF16, tag="pvn", bufs=2,
                            name=f"pvn{p}{nb}")
            den2 = work.tile([1, 2, 512], F32, tag="den2", bufs=1,
                             name=f"den2{p}{nb}")
            # the nb=1 sweep is ACT-paced (no projections left), so the
            # den copy moves to DVE there
            if nb == 0:
                nc.scalar.copy(out=den2.rearrange("p h q -> p (h q)"),
                               in_=pv[64:65, :, :].rearrange("p h q -> p (h q)"))
            else:
                nc.vector.tensor_copy(
                    out=den2.rearrange("p h q -> p (h q)"),
                    in_=pv[64:65, :, :].rearrange("p h q -> p (h q)"))
            nc.scalar.copy(out=pvn[:, 0, :], in_=pv[0:64, 0, :])
            nc.vector.tensor_copy(out=pvn[:, 1, :], in_=pv[0:64, 1, :])

            def tail():
                h0 = 2 * p
                # ONE reciprocal for both heads' denominator rows
                rcp_f = work.tile([1, 2, 512], F32, tag="rcpf", bufs=1,
                                  name=f"rcpf{p}{nb}")
                nc.vector.reciprocal_approx_fast(
                    out=rcp_f.rearrange("p h q -> p (h q)"),
                    in_=den2.rearrange("p h q -> p (h q)"))
                if last:
                    # endgame: the PE is idle and this chain gates the
                    # nb=1 output projection - broadcast via K=1 matmuls
                    # instead of the slower gpsimd path
                    brs = []
                    for hh in range(2):
                        rcpr = work.tile([1, 512], F32R, tag="rcpr", bufs=1,
                                         name=f"rcpr{p}{nb}{hh}")
                        with nc.allow_low_precision(
                                reason="f32r denominator broadcast"):
                            nc.vector.tensor_copy(out=rcpr,
                                                  in_=rcp_f[:, hh, :])
                        br_ps = pp.tile([64, 512], F32, tag="pp",
                                        name=f"br{p}{nb}{hh}")
                        nc.tensor.matmul(br_ps[:, :], ones_r, rcpr,
                                         start=True, stop=True)
                        brs.append(br_ps)
                    for hh in range(2):
                        h = h0 + hh
                        nc.vector.scalar_tensor_tensor(
                            out=xT[h // 2][(h % 2) * 64:(h % 2) * 64 + 64,
                                           nb * 512:(nb + 1) * 512],
                            in0=pvn[:, hh, :],
                            scalar=svh_sb[0:64, h:h + 1],
                            in1=brs[hh][:, :], op0=ALU.add, op1=ALU.mult)
                    return
                # ONE gpsimd broadcast serves both heads
                br = work.tile([64, 2, 512], F32, tag="brs", bufs=1,
                               name=f"brs{p}{nb}")
                nc.gpsimd.partition_broadcast(
                    br.rearrange("p h q -> p (h q)"),
                    rcp_f.rearrange("p h q -> p (h q)"))
                for hh in range(2):
                    h = h0 + hh
                    nc.vector.scalar_tensor_tensor(
                        out=xT[h // 2][(h % 2) * 64:(h % 2) * 64 + 64,
                                       nb * 512:(nb + 1) * 512],
                        in0=pvn[:, hh, :], scalar=svh_sb[0:64, h:h + 1],
                        in1=br[:, hh, :], op0=ALU.add, op1=ALU.mult)

            pending_tail[0] = tail

        for nb in range(QNB):
            for p in range(NPAIR):
                run_iteration(
                    p, nb,
                    vproj=(nb == 0 and p == 0),
                    projpair=(p + 1 if (nb == 0 and p < NPAIR - 1) else None),
                    qb1pair=(nb == 0 and p == NPAIR - 1),
                    outp0=(nb == 1 and p in (1, 2)),
                    last=(nb == 1 and p == NPAIR - 1))
                if nb == 0 and p == 0:
                    # V inputs are dead once (0,0)'s vproj is emitted;
                    # reuse the space for the qb=1 queries + wob
                    vsp_cm.__exit__(None, None, None)
                    late = ctx.enter_context(tc.tile_pool(name="late", bufs=1))
                    qTf1 = late.tile([128, EC, 512], BF16, name="qTf1")
                    qTfs[1] = qTf1
                    nc.sync.dma_start(out=qTf1, in_=qT[:, 1])
                    wof = late.tile([128, HC, D], BF16, name="wof")
                    wob[0] = wof
                    nc.sync.dma_start(out=wof, in_=woT[:, :, :])
                if nb == 0 and p == 2:
                    # kq inputs/weights die after proj_pair(3) is emitted
                    kqd_cm.__exit__(None, None, None)
        pending_tail[0]()
        pending_tail[0] = None
        # endgame: alternate the bias-add evacuations across ACT/DVE so
        # both engines drain the output chains in parallel
        for dc in range(EC):
            emit_outproj_dc(1, dc, evac_act=(dc % 2 == 0))


def _get_program():
    global _PROGRAM
    if _PROGRAM is None:
        _PROGRAM = _build_program()
    return _PROGRAM


def kernel(q, k, v, mask, Wq, bq, Wk, bk, Wv, bv, Wo, bo, _trace=False):
    bf16 = ml_dtypes.bfloat16
    q = np.asarray(q, np.float32)
    k = np.asarray(k, np.float32)
    v = np.asarray(v, np.float32)
    Wq = np.asarray(Wq, np.float32)
    Wk = np.asarray(Wk, np.float32)
    Wv = np.asarray(Wv, np.float32)
    Wo = np.asarray(Wo, np.float32)
    bq = np.asarray(bq, np.float32)
    bk = np.asarray(bk, np.float32)
    bv = np.asarray(bv, np.float32)
    bo = np.asarray(bo, np.float32)
    mask_f = np.asarray(mask).astype(np.float32)

    nc = _get_program()

    def fold_ec(xT, inner):  # [D, N] -> [128, EC, N] views
        N = xT.shape[1]
        a = np.ascontiguousarray(xT.reshape(EC, 128, N).transpose(1, 0, 2))
        return a.astype(bf16).reshape((128,) + inner)

    # qT: [D, Q] -> [128, QNB, EC, 512] (query-block major)
    def fold_q(xT):
        a = xT.reshape(EC, 128, QNB, 512).transpose(1, 2, 0, 3)
        return np.ascontiguousarray(a).astype(bf16)

    # kT: [D, S] -> [128, SNB, EC, 512] (512-col block major)
    kT_b = [np.ascontiguousarray(
        k[b].T.reshape(EC, 128, SNB, 512).transpose(1, 2, 0, 3)).astype(bf16)
        for b in range(B)]
    # vT: [D, S] -> [128, 4, EC, 512] (quarter major)
    vT_b = [np.ascontiguousarray(
        v[b].T.reshape(EC, 128, 4, 512).transpose(1, 2, 0, 3)).astype(bf16)
        for b in range(B)]
    # mask un-duplicated: [128, KC, QNB, 512]
    def build_mask(mT):  # mT: [S, Q] 0/1 float
        a = mT.reshape(KC, 128, QNB, 512).transpose(1, 0, 2, 3)
        return np.ascontiguousarray(a).astype(bf16)
    mask_b = [[build_mask(mask_f[b, 0, sq * Q:(sq + 1) * Q, :].T)
               for sq in range(2)] for b in range(B)]
    # host-side V column sums: svh[0:64, h] = (sum_tok v) @ Wv_h + S*bv_h
    svh_b = []
    for b in range(B):
        vsum = v[b].sum(0)  # [D]
        per_g = []
        for hg in range(2):
            hsl_g = slice(hg * DH, (hg + 1) * DH)
            s = vsum @ Wv[hsl_g, :].T + S * bv[hsl_g]  # [DH]
            arr = np.zeros((65, 8), np.float32)
            arr[0:64, :] = s.reshape(8, 64).T
            arr[64, :] = S
            per_g.append(np.ascontiguousarray(arr))
        svh_b.append(per_g)

    in_maps = []
    for c in range(8):
        b, hg, sq = c // 4, (c // 2) % 2, c % 2
        hsl = slice(hg * DH, (hg + 1) * DH)
        in_maps.append({
            "qT": fold_q(q[b, sq * Q:(sq + 1) * Q, :].T),
            "kT": kT_b[b],
            "vT": vT_b[b],
            "maskT": mask_b[b][sq],
            "svhv": svh_b[b][hg],
            "wqT": fold_ec(Wq.T[:, hsl], (EC, DH)),
            "wkT": fold_ec(Wk.T[:, hsl], (EC, DH)),
            "wvT": fold_ec(Wv.T[:, hsl], (EC, DH)),
            "woT": np.ascontiguousarray(
                Wo.T[hsl, :].reshape(HC, 128, D).transpose(1, 0, 2)).astype(bf16),
            "bqv": np.ascontiguousarray(bq[hsl].reshape(NPAIR, 128).T),
            "bkv": np.ascontiguousarray(bk[hsl].reshape(NPAIR, 128).T),
            "bov": np.ascontiguousarray(
                ((bo if hg == 0 else np.zeros_like(bo))
                 + Wo[:, hsl] @ bv[hsl]).reshape(EC, 128).T),
        })

    kw = {}
    if _trace:
        kw = dict(trace=True, trace_cores=list(range(8)))
    res = run_bass_kernel_spmd(nc, in_maps, core_ids=list(range(8)), **kw)
    kernel._last_res = res

    outp = np.empty((B, S, D), np.float32)
    for b in range(B):
        for sq in range(2):
            c0 = b * 4 + sq
            c1 = b * 4 + 2 + sq
            outp[b, sq * Q:(sq + 1) * Q, :] = (
                res.results[c0]["out"] + res.results[c1]["out"]).T
    if _trace:
        return outp, res
    return outp


# revision 37
# speedup vs baseline: 1.0183x; 1.0144x over previous
"""Multi-head attention TRN2 kernel (B=2, S=2048, D=1024, H=16).

Sharding (8 cores): B(2) x head-group(2) x query-block(2).
Each core: one batch b, 8 heads, 1024 query rows. The output projection
is a per-head-group partial sum; the host adds the two partials while
gathering (unshard step).

All on-chip compute is bf16 (fp8 anywhere in the q/k/v/E path measures
~3.5% output error per projection - the attention output is itself an
average of ~unit values, so elementwise quantization noise does NOT
average down relative to the signal).

The kernel is DVE-occupancy-bound (each DVE op pays an init+drain
overhead of ~150-200ns on top of its streaming time), so the design
minimizes DVE op count and keeps every op in its fastest perf mode:
- the post-exp (E-1) and *mask run once per 4-chunk GROUP as
  [128,4096] ops: tensor_scalar at 4x mode + tensor_tensor at 2x,
  amortizing the per-op overhead ~4x;
- the mask is DMA'd UN-duplicated and read with a stride-0 broadcast
  AP over the 2-head dim (halves mask DMA and SBUF);
- the denominator "+S" is accumulated into the PSUM ones-row by a
  K=1 f32r matmul (kills 16 DVE bias-adds);
- both heads' PV accumulate into ONE [65,2,512] PSUM tile, so the
  reciprocal runs once per pair on [1,1024] and ONE gpsimd
  partition_broadcast serves both heads;
- PSUM evacuations are split ACT/DVE (ACT is ~20% cheaper per PSUM
  read and has slack between exps; startup evacuations all go to ACT
  which is otherwise idle there).

Iteration order is nb-major (all 4 pairs at nb=0, then nb=1): all
kh/qh stay resident, the nb=0 output projection overlaps the nb=1
sweep, and only the last pair's tail is on the drain path. The qb=1
query projections are deferred to the (3,0) iteration, and the qT
transfer is split so only the qb=0 half is on the startup critical
path.

Softmax: the reference masks scores with -1e-9 (sic), so masked
positions contribute exp(-1e-9) == 1.0f exactly. PV weights are
a = (exp(s/8) - 1) * m; the "+1 everywhere" plane is restored from
HOST-precomputed V column sums (svh = (sum_tok v) @ Wv + S*bv) and an
appended ones-column on V gives the mask-dependent part of the
denominator.
"""

import sys

if "/opt/trn_rl_repo" not in sys.path:
    sys.path.insert(0, "/opt/trn_rl_repo")

import numpy as np
import ml_dtypes

import concourse.bass as bass
import concourse.tile as tile
from concourse import bacc, mybir
from concourse.bass_utils import run_bass_kernel_spmd

F32 = mybir.dt.float32
F32R = mybir.dt.float32r
BF16 = mybir.dt.bfloat16
AF = mybir.ActivationFunctionType
ALU = mybir.AluOpType

B, S, D, H = 2, 2048, 1024, 16
DK = 64
Q = 1024          # query rows per core
DH = 512          # head-group feature dims per core
NPAIR = 4         # head pairs per core
KC = S // 128     # 16 contraction chunks over k tokens
NG = KC // 4      # 4 chunk-groups per iteration (DVE ops run per group)
EC = D // 128     # 8 contraction chunks over model dim
QNB = Q // 512    # 2 query n-blocks
SNB = S // 512    # 4 khT n-blocks
HC = DH // 128    # 4 xT partition chunks

_PROGRAM = None


def _build_program():
    nc = bacc.Bacc("TRN2", debug=False, num_devices=8)

    qT = nc.dram_tensor("qT", [128, QNB, EC, 512], BF16, kind="ExternalInput")
    kT = nc.dram_tensor("kT", [128, SNB, EC, 512], BF16, kind="ExternalInput")
    vT = nc.dram_tensor("vT", [128, 4, EC, 512], BF16, kind="ExternalInput")
    maskT = nc.dram_tensor("maskT", [128, KC, QNB, 512], BF16,
                           kind="ExternalInput")
    svhv = nc.dram_tensor("svhv", [65, 8], F32, kind="ExternalInput")
    wqT = nc.dram_tensor("wqT", [128, EC, DH], BF16, kind="ExternalInput")
    wkT = nc.dram_tensor("wkT", [128, EC, DH], BF16, kind="ExternalInput")
    wvT = nc.dram_tensor("wvT", [128, EC, DH], BF16, kind="ExternalInput")
    woT = nc.dram_tensor("woT", [128, HC, D], BF16, kind="ExternalInput")
    bqv = nc.dram_tensor("bqv", [128, NPAIR], F32, kind="ExternalInput")
    bkv = nc.dram_tensor("bkv", [128, NPAIR], F32, kind="ExternalInput")
    bov = nc.dram_tensor("bov", [128, EC], F32, kind="ExternalInput")
    out = nc.dram_tensor("out", [D, Q], F32, kind="ExternalOutput")

    with tile.TileContext(nc) as tc:
        _emit(nc, tc, qT, kT, vT, maskT, wqT, wkT, wvT, woT, bqv, bkv, bov,
              svhv, out)
    nc.compile()
    return nc


def _emit(nc, tc, qT, kT, vT, maskT, wqT, wkT, wvT, woT, bqv, bkv, bov, svhv,
          out):
    from contextlib import ExitStack

    ctx = ExitStack()
    with ctx:
        consts = ctx.enter_context(tc.tile_pool(name="consts", bufs=1))
        big = ctx.enter_context(tc.tile_pool(name="big", bufs=1))
        work = ctx.enter_context(tc.tile_pool(name="work", bufs=2))
        pp = ctx.enter_context(tc.tile_pool(name="pp", bufs=2, space="PSUM"))
        pvp = ctx.enter_context(tc.tile_pool(name="pvp", bufs=1, space="PSUM"))
        scp = ctx.enter_context(tc.tile_pool(name="scp", bufs=2, space="PSUM"))
        kqd_cm = tc.tile_pool(name="kqd", bufs=1, side="right")
        kqd = kqd_cm.__enter__()
        vsp_cm = tc.tile_pool(name="vsp", bufs=1)
        vsp = vsp_cm.__enter__()

        # ---- small constants ----
        t_bq = consts.tile([128, NPAIR], F32)
        t_bk = consts.tile([128, NPAIR], F32)
        t_bo = consts.tile([128, EC], F32)
        nc.gpsimd.dma_start(out=t_bq, in_=bqv[:, :])
        nc.gpsimd.dma_start(out=t_bk, in_=bkv[:, :])
        nc.gpsimd.dma_start(out=t_bo, in_=bov[:, :])
        svh_sb = consts.tile([65, 8], F32)
        nc.gpsimd.dma_start(out=svh_sb, in_=svhv[:, :])
        ones_f0 = consts.tile([1, 64], F32)
        nc.vector.memset(ones_f0, 1.0)
        ones_r = consts.tile([1, 64], F32R)
        with nc.allow_low_precision(reason="f32r ones for denominator broadcast"):
            nc.vector.tensor_copy(out=ones_r, in_=ones_f0)
        # the K=1 "+S" matmul operands: [0..0,S](1,65) @ ones(1,512) -
        # rows 0:64 accumulate +0 (harmless), the ones-row gets +S
        ones512_f = consts.tile([1, 512], F32)
        nc.vector.memset(ones512_f, 1.0)
        ones512_r = consts.tile([1, 512], F32R)
        sv_f = consts.tile([1, 65], F32)
        nc.vector.memset(sv_f, 0.0)
        nc.vector.memset(sv_f[:, 64:65], float(S))
        sv_r = consts.tile([1, 65], F32R)
        with nc.allow_low_precision(reason="f32r +S denominator matmul"):
            nc.vector.tensor_copy(out=ones512_r, in_=ones512_f)
            nc.vector.tensor_copy(out=sv_r, in_=sv_f)
        # warm the ACT exp table set (~2.7us load) while ACT is idle at
        # startup instead of on the first real exp of the attention loop
        warm = consts.tile([1, 16], F32)
        nc.vector.memset(warm, 0.0)
        warm2 = consts.tile([1, 16], BF16)
        nc.scalar.activation(out=warm2, in_=warm, func=AF.Exp, scale=1.0)

        # ---- resident activation tensors (all pairs live: nb-major) ----
        khTs = {p: big.tile([128, S], BF16, name=f"khT{p}") for p in range(NPAIR)}
        qhTs = {p: big.tile([128, Q], BF16, name=f"qhT{p}") for p in range(NPAIR)}
        # one big vh tile: ONE memset covers every ones-column
        vh_aug = big.tile([128, KC, 8, 65], BF16, name="vha")
        nc.vector.memset(vh_aug, 1.0)
        # mask un-duplicated; the group (E-1)*m op reads it with a
        # stride-0 broadcast over the 2-head dim
        mbt = big.tile([128, KC, QNB, 1, 512], BF16, name="mbt")
        xT = [big.tile([128, Q], BF16, name=f"xT{i}") for i in range(HC)]

        wvf = vsp.tile([128, EC, DH], BF16, name="wvf")
        kTf = kqd.tile([128, SNB, EC, 512], BF16, name="kTf")
        qTf0 = kqd.tile([128, EC, 512], BF16, name="qTf0")
        qTfs = [qTf0, None]
        wkf = kqd.tile([128, EC, DH], BF16, name="wkf")
        wqf = kqd.tile([128, EC, DH], BF16, name="wqf")

        # V input quarters rotate through 2 buffers (quarters 0/1 are
        # consumed at startup, 2/3 inside iteration (0,0)); their DMAs
        # ride the gpsimd queue so the WAR wait on buffer reuse doesn't
        # stall the mask transfers on the sync queue.
        vqs = [vsp.tile([128, EC, 512], BF16, tag="vq", bufs=2, name=f"vq{t}")
               for t in range(4)]

        # inputs in PE-need order; transfers serialize per queue, so
        # queue order IS arrival order
        # two hw DMA queues, balanced ~9MB/7MB, ordered by PE need time;
        # vq quarter 3 reuses quarter 1's buffer, and its WAR wait rides
        # the scalar queue where the later masks can afford the stall
        nc.sync.dma_start(out=wkf, in_=wkT[:, :, :])
        nc.sync.dma_start(out=kTf[:, 0], in_=kT[:, 0])
        nc.sync.dma_start(out=kTf[:, 1], in_=kT[:, 1])
        nc.sync.dma_start(out=kTf[:, 2], in_=kT[:, 2])
        nc.sync.dma_start(out=wvf, in_=wvT[:, :, :])
        nc.sync.dma_start(out=vqs[0], in_=vT[:, 0])
        nc.sync.dma_start(out=kTf[:, 3], in_=kT[:, 3])
        nc.sync.dma_start(out=vqs[1], in_=vT[:, 1])
        nc.sync.dma_start(out=vqs[2], in_=vT[:, 2])
        nc.scalar.dma_start(out=wqf, in_=wqT[:, :, :])
        nc.scalar.dma_start(out=qTf0, in_=qT[:, 0])
        nc.scalar.dma_start(out=mbt[:, 0:4, :, 0, :], in_=maskT[:, 0:4])
        nc.scalar.dma_start(out=mbt[:, 4:8, :, 0, :], in_=maskT[:, 4:8])
        nc.scalar.dma_start(out=vqs[3], in_=vT[:, 3])
        nc.scalar.dma_start(out=mbt[:, 8:12, :, 0, :], in_=maskT[:, 8:12])
        nc.scalar.dma_start(out=mbt[:, 12:16, :, 0, :], in_=maskT[:, 12:16])

        def proj_tile(p, j, evac_act):
            if j < SNB:
                sb = j
                ps = pp.tile([128, 512], F32, tag="pp", name=f"khps{p}_{sb}")
                for ec in range(EC):
                    nc.tensor.matmul(
                        ps[:, :], wkf[:, ec, p * 128:(p + 1) * 128],
                        kTf[:, sb, ec, :],
                        start=(ec == 0), stop=(ec == EC - 1))
                dst = khTs[p][:, sb * 512:(sb + 1) * 512]
                bias = t_bk[:, p:p + 1]
            else:
                qb = j - SNB
                ps = pp.tile([128, 512], F32, tag="pp", name=f"qhps{p}_{qb}")
                for ec in range(EC):
                    nc.tensor.matmul(
                        ps[:, :], wqf[:, ec, p * 128:(p + 1) * 128],
                        qTfs[qb][:, ec, :],
                        start=(ec == 0), stop=(ec == EC - 1))
                dst = qhTs[p][:, qb * 512:(qb + 1) * 512]
                bias = t_bq[:, p:p + 1]
            if evac_act:
                nc.scalar.activation(out=dst, in_=ps[:, :], func=AF.Identity,
                                     bias=bias, scale=1.0)
            else:
                nc.vector.tensor_scalar(
                    out=dst, in0=ps[:, :], scalar1=bias, scalar2=None,
                    op0=ALU.add)

        def emit_vproj_qtr(qtr, evac_act):
            vqf = vqs[qtr]
            for si in range(4):
                sc = qtr * 4 + si
                ps = pp.tile([128, 512], F32, tag="pp", name=f"vps{sc}")
                for ec in range(EC):
                    nc.tensor.matmul(
                        ps[:, :], vqf[:, ec, si * 128:(si + 1) * 128],
                        wvf[:, ec, :],
                        start=(ec == 0), stop=(ec == EC - 1))
                src = ps.rearrange("p (h d) -> p h d", h=8)
                dst = vh_aug[:, sc, :, 0:64]
                if evac_act:
                    nc.scalar.copy(out=dst, in_=src)
                else:
                    nc.vector.tensor_copy(out=dst, in_=src)

        # startup: the minimal prefix for iteration (0,0) - pair-0
        # projections and V quarter 0 in DMA arrival order (quarters
        # 1..3 are emitted inside (0,0)'s first groups); ACT is idle
        # here so it takes every evac
        proj_tile(0, 0, True)
        proj_tile(0, 4, True)
        proj_tile(0, 1, True)
        proj_tile(0, 2, True)
        emit_vproj_qtr(0, evac_act=True)
        proj_tile(0, 3, True)

        # ---- attention ----
        pending_tail = [None]
        wob = [None]

        def emit_outproj_dc(nb, dc, evac_act):
            ps = pp.tile([128, 512], F32, tag="pp", name=f"ops{dc}_{nb}")
            for hc in range(HC):
                nc.tensor.matmul(
                    ps[:, :], wob[0][:, hc, dc * 128:(dc + 1) * 128],
                    xT[hc][:, nb * 512:(nb + 1) * 512],
                    start=(hc == 0), stop=(hc == HC - 1))
            o_sb = work.tile([128, 512], F32, tag="osb", bufs=2,
                             name=f"osb{dc}_{nb}")
            if evac_act:
                nc.scalar.add(out=o_sb, in_=ps[:, :], add=t_bo[:, dc:dc + 1])
            else:
                nc.vector.tensor_scalar(
                    out=o_sb, in0=ps[:, :], scalar1=t_bo[:, dc:dc + 1],
                    scalar2=None, op0=ALU.add)
            nc.sync.dma_start(
                out=out[dc * 128:(dc + 1) * 128, nb * 512:(nb + 1) * 512],
                in_=o_sb)

        def run_iteration(p, nb, vproj=False, projpair=None, qb1pair=False,
                          outp0=False, last=False):
            # both heads accumulate into ONE 2-bank PSUM tile; row 64 is
            # the ones-column (denominator) row
            pv = pvp.tile([65, 2, 512], F32, tag="pv", name=f"pv_{p}{nb}")
            EQs = {}

            def emit_scores(kc, EQ):
                sc_ps = scp.tile([128, 2, 512], F32, tag="sc",
                                 name=f"sc_{p}_{nb}_{kc}")
                nc.tensor.matmul(
                    sc_ps[:, 0, :],
                    khTs[p][0:64, kc * 128:(kc + 1) * 128],
                    qhTs[p][0:64, nb * 512:(nb + 1) * 512],
                    start=True, stop=True)
                nc.tensor.matmul(
                    sc_ps[:, 1, :],
                    khTs[p][64:128, kc * 128:(kc + 1) * 128],
                    qhTs[p][64:128, nb * 512:(nb + 1) * 512],
                    start=True, stop=True, tile_position=(64, 0))
                # exp straight from PSUM into this chunk's slot of the
                # group tile
                nc.scalar.activation(
                    out=EQ[:, kc % 4].rearrange("p h q -> p (h q)"),
                    in_=sc_ps.rearrange("p h q -> p (h q)"),
                    func=AF.Exp, scale=0.125)

            def emit_group_mask(g, EQ, half):
                # (E-1) at DVE 4x + *mask at 2x, in 2-chunk halves so the
                # first PV pair of a group unblocks right after its 2 exps
                sl = EQ[:, 2 * half:2 * half + 2]
                slf = sl.rearrange("a b h q -> a (b h q)")
                nc.vector.tensor_scalar(
                    out=slf, in0=slf, scalar1=-1.0, scalar2=None, op0=ALU.add)
                c = 4 * g + 2 * half
                nc.vector.tensor_tensor(
                    out=sl, in0=sl,
                    in1=mbt[:, c:c + 2, nb].to_broadcast([128, 2, 2, 512]),
                    op=ALU.mult)

            def emit_pv(kc, EQ):
                # pv row 0 is the ones-column (denominator) row so the
                # reciprocal later reads a partition-0 AP (the custom
                # DVE ucode ignores nonzero base partitions)
                for hh in range(2):
                    nc.tensor.matmul(
                        pv[:, hh, :], vh_aug[:, kc, 2 * p + hh, :],
                        EQ[:, kc % 4, hh, :],
                        start=(kc == 0), stop=False)

            # per-group pipeline: scores+exp of group g interleaved (in
            # PE order) with PV of g-1, so the PE always has PV work
            # while it waits on score-PSUM banks; the DVE mask ops run
            # in 2-chunk halves behind the exps
            for g in range(NG + 1):
                EQ = None
                if g < NG:
                    EQ = work.tile([128, 4, 2, 512], BF16, tag="EQ", bufs=2,
                                   name=f"EQ{p}_{nb}_{g}")
                    if vproj and g in (0, 1, 2):
                        emit_vproj_qtr(1 + g, evac_act=(g == 0))
                EQprev = EQs.pop(g - 1) if g >= 1 else None
                for i in range(4):
                    if g < NG:
                        emit_scores(4 * g + i, EQ)
                        if i == 1:
                            emit_group_mask(g, EQ, 0)
                        elif i == 3:
                            emit_group_mask(g, EQ, 1)
                    if g == 1 and i == 0 and pending_tail[0] is not None:
                        pending_tail[0]()
                        pending_tail[0] = None
                    if EQprev is not None:
                        emit_pv(4 * (g - 1) + i, EQprev)
                if g < NG:
                    EQs[g] = EQ
                if projpair is not None and 1 <= g <= 3:
                    # 5 tiles (kh x4 + qh qb0) spread over groups 1..3 -
                    # group 4 stays projection-free so the iteration
                    # boundary (last PV group + next scores) clears fast
                    for j in {1: [0, 1], 2: [2, 3], 3: [4]}[g]:
                        proj_tile(projpair, j, evac_act=(j % 2 == 0))
                if qb1pair and g <= 3:
                    # deferred qb=1 projections for all pairs
                    proj_tile(g, 5, evac_act=(g % 2 == 0))
                if outp0 and 1 <= g:
                    emit_outproj_dc(0, (0 if p == 1 else 4) + g - 1,
                                    evac_act=False)

            # close the accumulation with the "+S" ones-row matmuls, then
            # evacuate both heads (ACT + DVE) so the pv banks free fast
            for hh in range(2):
                nc.tensor.matmul(
                    pv[:, hh, :], sv_r, ones512_r,
                    start=False, stop=True, skip_group_check=True)
            # the denominator rows go to a partition-0 tile (the custom
            # reciprocal ucode ignores nonzero base partitions) via ACT,
            # which reads PSUM cheaply
            pvn = work.tile([64, 2, 512], BF16, tag="pvn", bufs=2,
                            name=f"pvn{p}{nb}")
            den2 = work.tile([1, 2, 512], F32, tag="den2", bufs=1,
                             name=f"den2{p}{nb}")
            nc.scalar.copy(out=den2.rearrange("p h q -> p (h q)"),
                           in_=pv[64:65, :, :].rearrange("p h q -> p (h q)"))
            nc.scalar.copy(out=pvn[:, 0, :], in_=pv[0:64, 0, :])
            nc.vector.tensor_copy(out=pvn[:, 1, :], in_=pv[0:64, 1, :])

            def tail():
                h0 = 2 * p
                # ONE reciprocal for both heads' denominator rows
                rcp_f = work.tile([1, 2, 512], F32, tag="rcpf", bufs=1,
                                  name=f"rcpf{p}{nb}")
                nc.vector.reciprocal_approx_fast(
                    out=rcp_f.rearrange("p h q -> p (h q)"),
                    in_=den2.rearrange("p h q -> p (h q)"))
                if last:
                    # endgame: the PE is idle and this chain gates the
                    # nb=1 output projection - broadcast via K=1 matmuls
                    # instead of the slower gpsimd path
                    brs = []
                    for hh in range(2):
                        rcpr = work.tile([1, 512], F32R, tag="rcpr", bufs=1,
                                         name=f"rcpr{p}{nb}{hh}")
                        with nc.allow_low_precision(
                                reason="f32r denominator broadcast"):
                            nc.vector.tensor_copy(out=rcpr,
                                                  in_=rcp_f[:, hh, :])
                        br_ps = pp.tile([64, 512], F32, tag="pp",
                                        name=f"br{p}{nb}{hh}")
                        nc.tensor.matmul(br_ps[:, :], ones_r, rcpr,
                                         start=True, stop=True)
                        brs.append(br_ps)
                    for hh in range(2):
                        h = h0 + hh
                        nc.vector.scalar_tensor_tensor(
                            out=xT[h // 2][(h % 2) * 64:(h % 2) * 64 + 64,
                                           nb * 512:(nb + 1) * 512],
                            in0=pvn[:, hh, :],
                            scalar=svh_sb[0:64, h:h + 1],
                            in1=brs[hh][:, :], op0=ALU.add, op1=ALU.mult)
                    return
                # ONE gpsimd broadcast serves both heads
                br = work.tile([64, 2, 512], F32, tag="brs", bufs=1,
                               name=f"brs{p}{nb}")
                nc.gpsimd.partition_broadcast(
                    br.rearrange("p h q -> p (h q)"),
                    rcp_f.rearrange("p h q -> p (h q)"))
                for hh in range(2):
                    h = h0 + hh
                    nc.vector.scalar_tensor_tensor(
                        out=xT[h // 2][(h % 2) * 64:(h % 2) * 64 + 64,
                                       nb * 512:(nb + 1) * 512],
                        in0=pvn[:, hh, :], scalar=svh_sb[0:64, h:h + 1],
                        in1=br[:, hh, :], op0=ALU.add, op1=ALU.mult)

            pending_tail[0] = tail

        for nb in range(QNB):
            for p in range(NPAIR):
                run_iteration(
                    p, nb,
                    vproj=(nb == 0 and p == 0),
                    projpair=(p + 1 if (nb == 0 and p < NPAIR - 1) else None),
                    qb1pair=(nb == 0 and p == NPAIR - 1),
                    outp0=(nb == 1 and p in (1, 2)),
                    last=(nb == 1 and p == NPAIR - 1))
                if nb == 0 and p == 0:
                    # V inputs are dead once (0,0)'s vproj is emitted;
                    # reuse the space for the qb=1 queries + wob
                    vsp_cm.__exit__(None, None, None)
                    late = ctx.enter_context(tc.tile_pool(name="late", bufs=1))
                    qTf1 = late.tile([128, EC, 512], BF16, name="qTf1")
                    qTfs[1] = qTf1
                    nc.sync.dma_start(out=qTf1, in_=qT[:, 1])
                    wof = late.tile([128, HC, D], BF16, name="wof")
                    wob[0] = wof
                    nc.sync.dma_start(out=wof, in_=woT[:, :, :])
                if nb == 0 and p == 2:
                    # kq inputs/weights die after proj_pair(3) is emitted
                    kqd_cm.__exit__(None, None, None)
        pending_tail[0]()
        pending_tail[0] = None
        for dc in range(EC):
            emit_outproj_dc(1, dc, evac_act=True)


def _get_program():
    global _PROGRAM
    if _PROGRAM is None:
        _PROGRAM = _build_program()
    return _PROGRAM


def kernel(q, k, v, mask, Wq, bq, Wk, bk, Wv, bv, Wo, bo, _trace=False):
    bf16 = ml_dtypes.bfloat16
    q = np.asarray(q, np.float32)
    k = np.asarray(k, np.float32)
    v = np.asarray(v, np.float32)
    Wq = np.asarray(Wq, np.float32)
    Wk = np.asarray(Wk, np.float32)
    Wv = np.asarray(Wv, np.float32)
    Wo = np.asarray(Wo, np.float32)
    bq = np.asarray(bq, np.float32)
    bk = np.asarray(bk, np.float32)
    bv = np.asarray(bv, np.float32)
    bo = np.asarray(bo, np.float32)
    mask_f = np.asarray(mask).astype(np.float32)

    nc = _get_program()

    def fold_ec(xT, inner):  # [D, N] -> [128, EC, N] views
        N = xT.shape[1]
        a = np.ascontiguousarray(xT.reshape(EC, 128, N).transpose(1, 0, 2))
        return a.astype(bf16).reshape((128,) + inner)

    # qT: [D, Q] -> [128, QNB, EC, 512] (query-block major)
    def fold_q(xT):
        a = xT.reshape(EC, 128, QNB, 512).transpose(1, 2, 0, 3)
        return np.ascontiguousarray(a).astype(bf16)

    # kT: [D, S] -> [128, SNB, EC, 512] (512-col block major)
    kT_b = [np.ascontiguousarray(
        k[b].T.reshape(EC, 128, SNB, 512).transpose(1, 2, 0, 3)).astype(bf16)
        for b in range(B)]
    # vT: [D, S] -> [128, 4, EC, 512] (quarter major)
    vT_b = [np.ascontiguousarray(
        v[b].T.reshape(EC, 128, 4, 512).transpose(1, 2, 0, 3)).astype(bf16)
        for b in range(B)]
    # mask un-duplicated: [128, KC, QNB, 512]
    def build_mask(mT):  # mT: [S, Q] 0/1 float
        a = mT.reshape(KC, 128, QNB, 512).transpose(1, 0, 2, 3)
        return np.ascontiguousarray(a).astype(bf16)
    mask_b = [[build_mask(mask_f[b, 0, sq * Q:(sq + 1) * Q, :].T)
               for sq in range(2)] for b in range(B)]
    # host-side V column sums: svh[0:64, h] = (sum_tok v) @ Wv_h + S*bv_h
    svh_b = []
    for b in range(B):
        vsum = v[b].sum(0)  # [D]
        per_g = []
        for hg in range(2):
            hsl_g = slice(hg * DH, (hg + 1) * DH)
            s = vsum @ Wv[hsl_g, :].T + S * bv[hsl_g]  # [DH]
            arr = np.zeros((65, 8), np.float32)
            arr[0:64, :] = s.reshape(8, 64).T
            arr[64, :] = S
            per_g.append(np.ascontiguousarray(arr))
        svh_b.append(per_g)

    in_maps = []
    for c in range(8):
        b, hg, sq = c // 4, (c // 2) % 2, c % 2
        hsl = slice(hg * DH, (hg + 1) * DH)
        in_maps.append({
            "qT": fold_q(q[b, sq * Q:(sq + 1) * Q, :].T),
            "kT": kT_b[b],
            "vT": vT_b[b],
            "maskT": mask_b[b][sq],
            "svhv": svh_b[b][hg],
            "wqT": fold_ec(Wq.T[:, hsl], (EC, DH)),
            "wkT": fold_ec(Wk.T[:, hsl], (EC, DH)),
            "wvT": fold_ec(Wv.T[:, hsl], (EC, DH)),
            "woT": np.ascontiguousarray(
                Wo.T[hsl, :].reshape(HC, 128, D).transpose(1, 0, 2)).astype(bf16),
            "bqv": np.ascontiguousarray(bq[hsl].reshape(NPAIR, 128).T),
            "bkv": np.ascontiguousarray(bk[hsl].reshape(NPAIR, 128).T),
            "bov": np.ascontiguousarray(
                ((bo if hg == 0 else np.zeros_like(bo))
                 + Wo[:, hsl] @ bv[hsl]).reshape(EC, 128).T),
        })

    kw = {}
    if _trace:
        kw = dict(trace=True, trace_cores=list(range(8)))
    res = run_bass_kernel_spmd(nc, in_maps, core_ids=list(range(8)), **kw)
    kernel._last_res = res

    outp = np.empty((B, S, D), np.float32)
    for b in range(B):
        for sq in range(2):
            c0 = b * 4 + sq
            c1 = b * 4 + 2 + sq
            outp[b, sq * Q:(sq + 1) * Q, :] = (
                res.results[c0]["out"] + res.results[c1]["out"]).T
    if _trace:
        return outp, res
    return outp
